# revision 14
# baseline (speedup 1.0000x reference)
"""Ernie4 MoE (T=2048, H=1024, E=64 top-6, I=512 + shared SwiGLU MLP) on 8 Trainium2 cores.

Strategy: expert parallelism with host-mediated all-to-all.
  * Host computes the router (gate logits, sigmoid, top-6, renormalized combine
    weights) in fp32 and performs the dispatch: experts are ranked by routed
    token count and dealt round-robin to the 8 cores (rank r -> core r%8,
    slot r//8) so per-slot widths are balanced; the SPMD program uses slot
    widths W[s] = max token count over cores at slot s (16-aligned). Each
    core receives a pre-gathered, pre-transposed activation block
    xgT[H, sum(W)] in bf16 plus its 8 experts' weights in bf16, prepacked
    per slot into a single contiguous [128, 24*512] SBUF-layout blob so each
    slot's weights stream as ONE 3MB DMA.
  * Device (per core, same program): shared SwiGLU MLP tensor-parallel over
    the SI dim (slice of 128), plus 8 expert SwiGLU FFNs on exact slot
    widths - dense bf16 matmuls only, no on-device routing/gather/scatter.
    Weight blobs alternate scalar/vector DMA queues, activations ride the
    sync queue, outputs drain on the gpsimd queue, so everything overlaps.
  * Outputs (shared partial [H,T] and expert block [H,sum(W)], both bf16)
    are combined on host: out = sum_c shared_c.T + weighted scatter of y.
"""

import numpy as np
import ml_dtypes

T, H, E, K, I, SI = 2048, 1024, 64, 6, 512, 1024
NCORE = 8
EC = E // NCORE          # expert slots per core
SIC = SI // NCORE        # shared-intermediate slice per core
KC = H // 128            # hidden-dim 128-chunks
ICN = I // 128           # expert-intermediate 128-chunks
BF16 = ml_dtypes.bfloat16

_CACHE = {}


def _build(W):
    import concourse.bass as bass
    import concourse.tile as tile
    from concourse import bacc, mybir

    f32 = mybir.dt.float32
    b16 = mybir.dt.bfloat16
    AF = mybir.ActivationFunctionType
    OP = mybir.AluOpType

    WTOT = sum(W)
    Wmax = max(W)
    assert Wmax <= 512
    offs = np.concatenate([[0], np.cumsum(W)]).astype(int)
    WBUF = 4  # weight-blob prefetch depth (slots in flight)

    nc = bacc.Bacc("TRN2", target_bir_lowering=False, debug=False,
                   enable_asserts=False, num_devices=NCORE)

    # all activation inputs are prepacked on host into SBUF chunk layout
    # ([128 partitions, contiguous free bytes]) so each load is one DMA
    # with 128 fully-contiguous descriptors.
    xsl = nc.dram_tensor("xsl", [4, 128, KC * 512], b16, kind="ExternalInput").ap()
    xg = nc.dram_tensor("xg", [128, KC * WTOT], b16, kind="ExternalInput").ap()
    wall = nc.dram_tensor("wall", [EC, 128, 24 * 512], b16, kind="ExternalInput").ap()
    wsh = nc.dram_tensor("wsh", [128, 3, 1024], b16, kind="ExternalInput").ap()
    outp_sh = nc.dram_tensor("outp_sh", [H, T], b16, kind="ExternalOutput").ap()
    yout = nc.dram_tensor("yout", [H, WTOT], b16, kind="ExternalOutput").ap()

    with tile.TileContext(nc) as tc:
        with (
            tc.tile_pool(name="consts", bufs=1) as consts,
            tc.tile_pool(name="wpool", bufs=WBUF) as wpool,
            tc.tile_pool(name="xtpool", bufs=2) as xtpool,
            tc.tile_pool(name="spool", bufs=4) as spool,
            tc.tile_pool(name="apool", bufs=2) as apool,
            tc.tile_pool(name="ypool", bufs=2) as ypool,
            tc.tile_pool(name="shpool", bufs=2) as shpool,
            tc.tile_pool(name="ps", bufs=8, space="PSUM") as ps,
        ):
            # ---- expert weight blobs: one 3MB DMA per slot (scalar queue) ----
            wtiles = []

            def emit_wdma(s):
                wt = wpool.tile([128, 24, 512], b16, tag="wall")
                nc.scalar.dma_start(wt[:], wall[s].rearrange("p (a b) -> p a b", b=512))
                wtiles.append(wt)

            for s in range(WBUF):
                emit_wdma(s)

            # weight views inside a slot blob
            def wg_v(wt, kc, ic):
                return wt[:, kc, ic * 128:(ic + 1) * 128]

            def wu_v(wt, kc, ic):
                return wt[:, 8 + kc, ic * 128:(ic + 1) * 128]

            def wd_v(wt, ic, hc):
                return wt[:, 16 + 2 * ic + hc // 4, (hc % 4) * 128:(hc % 4) * 128 + 128]

            # ---- per-slot routed-activation tiles (sync queue) ----
            xg_sbs = [consts.tile([128, KC, W[s]], b16, name=f"xg{s}")
                      for s in range(EC)]

            def emit_xg(s):
                nc.sync.dma_start(
                    xg_sbs[s][:],
                    xg[:, KC * offs[s]:KC * offs[s + 1]].rearrange(
                        "p (kc w) -> p kc w", w=W[s]))

            emit_xg(0)
            emit_xg(1)
            wsh_sb = consts.tile([128, 3, 1024], b16)
            nc.sync.dma_start(wsh_sb[:], wsh)
            a_s = consts.tile([128, T], b16)

            def emit_shared_gateup(sl):
                xts = xtpool.tile([128, KC, 512], b16, tag="xts")
                nc.sync.dma_start(xts[:], xsl[sl].rearrange("p (kc t) -> p kc t", t=512))
                pg = ps.tile([128, 512], f32, tag="ps")
                for kc in range(KC):
                    nc.tensor.matmul(pg[:], wsh_sb[:, 0, kc * 128:(kc + 1) * 128],
                                     xts[:, kc, :], start=(kc == 0), stop=(kc == KC - 1))
                pu = ps.tile([128, 512], f32, tag="ps")
                for kc in range(KC):
                    nc.tensor.matmul(pu[:], wsh_sb[:, 1, kc * 128:(kc + 1) * 128],
                                     xts[:, kc, :], start=(kc == 0), stop=(kc == KC - 1))
                sg = spool.tile([128, 512], f32, tag="sg")
                nc.scalar.activation(sg[:], pg[:], AF.Silu)
                nc.vector.tensor_tensor(a_s[:, sl * 512:(sl + 1) * 512], sg[:], pu[:], op=OP.mult)

            def emit_shared_down(sl):
                ysh = shpool.tile([128, KC, 512], b16, tag="ysh")
                for hc in range(KC):
                    py = ps.tile([128, 512], f32, tag="ps")
                    nc.tensor.matmul(py[:], wsh_sb[:, 2, hc * 128:(hc + 1) * 128],
                                     a_s[:, sl * 512:(sl + 1) * 512], start=True, stop=True)
                    if hc % 2 == 0:
                        nc.vector.tensor_copy(ysh[:, hc, :], py[:])
                    else:
                        nc.scalar.activation(ysh[:, hc, :], py[:], AF.Copy)
                nc.gpsimd.dma_start(
                    outp_sh.rearrange("(kc p) t -> p kc t", p=128)[:, :, sl * 512:(sl + 1) * 512],
                    ysh[:])

            def emit_expert(s):
                w = W[s]
                wt = wtiles[s]
                xgs = xg_sbs[s]
                a_sb = apool.tile([128, ICN, Wmax], b16, tag="a")
                for ic in range(ICN):
                    pg = ps.tile([128, 512], f32, tag="ps")
                    for kc in range(KC):
                        nc.tensor.matmul(pg[:, 0:w], wg_v(wt, kc, ic), xgs[:, kc, :],
                                         start=(kc == 0), stop=(kc == KC - 1))
                    pu = ps.tile([128, 512], f32, tag="ps")
                    for kc in range(KC):
                        nc.tensor.matmul(pu[:, 0:w], wu_v(wt, kc, ic), xgs[:, kc, :],
                                         start=(kc == 0), stop=(kc == KC - 1))
                    sg = spool.tile([128, 512], f32, tag="sg")
                    nc.scalar.activation(sg[:, 0:w], pg[:, 0:w], AF.Silu)
                    nc.vector.tensor_tensor(a_sb[:, ic, 0:w], sg[:, 0:w], pu[:, 0:w], op=OP.mult)

                y_sb = ypool.tile([128, KC, Wmax], b16, tag="y")
                for hc in range(KC):
                    py = ps.tile([128, 512], f32, tag="ps")
                    for ic in range(ICN):
                        nc.tensor.matmul(py[:, 0:w], wd_v(wt, ic, hc), a_sb[:, ic, 0:w],
                                         start=(ic == 0), stop=(ic == ICN - 1))
                    if hc % 2 == 0:
                        nc.vector.tensor_copy(y_sb[:, hc, 0:w], py[:, 0:w])
                    else:
                        nc.scalar.activation(y_sb[:, hc, 0:w], py[:, 0:w], AF.Copy)
                nc.gpsimd.dma_start(
                    yout.rearrange("(kc p) w -> p kc w", p=128)[:, :, offs[s]:offs[s] + w],
                    y_sb[:, :, 0:w])
                # next weight blob: emitted AFTER this slot's compute so the
                # scalar engine's wait (tile release of slot s) cannot block
                # the silu ops the PE needs for slot s itself.
                if s + WBUF < EC:
                    emit_wdma(s + WBUF)

            # ---- schedule: experts 0-1 first (their weights land first and
            # their completion releases buffers for blobs 4-5), shared MLP as
            # PE filler while blobs stream, then experts 2-7. ----
            emit_expert(0)
            for sl in range(4):
                emit_shared_gateup(sl)
            emit_expert(1)
            for s in range(2, EC):
                emit_xg(s)
            for sl in range(4):
                emit_shared_down(sl)
            for s in range(2, EC):
                emit_expert(s)

    nc.compile()
    return nc


def _get_nc(W):
    key = tuple(W)
    if key not in _CACHE:
        _CACHE[key] = _build(list(W))
    return _CACHE[key]


def _sbufize(m, nchunk):
    """[nchunk*128, F] row-major -> [128, nchunk*F] in SBUF chunk layout."""
    F = m.shape[1]
    return m.reshape(nchunk, 128, F).transpose(1, 0, 2).reshape(128, nchunk * F)


def _plan(inputs):
    x = np.ascontiguousarray(inputs["hidden_states"], dtype=np.float32)
    gate_w = np.asarray(inputs["gate_w"], dtype=np.float32)
    gate_bias = np.asarray(inputs["gate_bias"], dtype=np.float32)
    w_gate = np.asarray(inputs["w_gate"], dtype=np.float32)
    w_up = np.asarray(inputs["w_up"], dtype=np.float32)
    w_down = np.asarray(inputs["w_down"], dtype=np.float32)
    ws_gate = np.asarray(inputs["ws_gate"], dtype=np.float32)
    ws_up = np.asarray(inputs["ws_up"], dtype=np.float32)
    ws_down = np.asarray(inputs["ws_down"], dtype=np.float32)

    # ---- router (exact fp32, mirrors the reference) ----
    logits = x @ gate_w.T                                   # [T, E]
    scores = 1.0 / (1.0 + np.exp(-logits, dtype=np.float32))
    corrected = scores + gate_bias                          # [T, E]
    topk = np.argsort(-corrected, axis=1, kind="stable")[:, :K]  # [T, K]
    wsel = np.take_along_axis(scores, topk, axis=1)
    wsel = wsel / wsel.sum(axis=1, keepdims=True)

    toks = [None] * E
    cmbw = [None] * E
    sel = np.zeros((T, E), dtype=bool)
    sel[np.arange(T)[:, None], topk] = True
    wmat = np.zeros((T, E), dtype=np.float32)
    np.put_along_axis(wmat, topk, wsel, axis=1)
    for e in range(E):
        toks[e] = np.nonzero(sel[:, e])[0]
        cmbw[e] = wmat[toks[e], e]
    counts = np.array([len(t) for t in toks])

    # ---- balanced expert->(core,slot) assignment ----
    order = np.argsort(-counts, kind="stable")
    slot_expert = [[int(order[8 * s + c]) for s in range(EC)] for c in range(NCORE)]
    W = [max(16, int(-(-int(counts[order[8 * s]]) // 16) * 16)) for s in range(EC)]
    offs = np.concatenate([[0], np.cumsum(W)]).astype(int)
    WTOT = int(offs[-1])

    # xT in SBUF chunk layout [128, KC, T], then per-slab contiguous blocks
    xchunk = x.T.reshape(KC, 128, T).transpose(1, 0, 2)     # [128, KC, T]
    xsl = np.ascontiguousarray(
        xchunk.reshape(128, KC, 4, 512).transpose(2, 0, 1, 3).reshape(4, 128, KC * 512)
    ).astype(BF16)
    in_maps = []
    for c in range(NCORE):
        es = slot_expert[c]
        xg = np.zeros((128, KC * WTOT), dtype=np.float32)
        wall = np.empty((EC, 128, 24 * 512), dtype=np.float32)
        for s in range(EC):
            e = es[s]
            tk = toks[e]
            # per-slot block in SBUF chunk layout [128, KC*W[s]], contiguous
            blk = np.zeros((128, KC, W[s]), dtype=np.float32)
            xt = x[tk].T                                    # [H, cnt]
            blk[:, :, 0:len(tk)] = xt.reshape(KC, 128, len(tk)).transpose(1, 0, 2)
            xg[:, KC * offs[s]:KC * offs[s + 1]] = blk.reshape(128, -1)
            wall[s, :, 0:4096] = _sbufize(w_gate[e], KC)
            wall[s, :, 4096:8192] = _sbufize(w_up[e], KC)
            wall[s, :, 8192:12288] = _sbufize(w_down[e], ICN)
        wsh = np.stack([
            _sbufize(ws_gate[:, c * SIC:(c + 1) * SIC], KC),
            _sbufize(ws_up[:, c * SIC:(c + 1) * SIC], KC),
            ws_down[c * SIC:(c + 1) * SIC, :],
        ], axis=1)                                          # [128, 3, 1024]
        in_maps.append({
            "xsl": xsl,
            "xg": xg.astype(BF16),
            "wall": wall.astype(BF16),
            "wsh": wsh.astype(BF16),
        })
    return {"W": W, "offs": offs, "slot_expert": slot_expert, "toks": toks,
            "cmbw": cmbw, "in_maps": in_maps}


def _combine(plan, res):
    acc = np.zeros((T, H), dtype=np.float32)
    offs = plan["offs"]
    for c in range(NCORE):
        r = res.results[c]
        acc += np.asarray(r["outp_sh"]).astype(np.float32).T
        y = np.asarray(r["yout"])
        for s in range(EC):
            e = plan["slot_expert"][c][s]
            tk = plan["toks"][e]
            if len(tk) == 0:
                continue
            yb = y[:, offs[s]:offs[s] + len(tk)].astype(np.float32).T
            acc[tk, :] += plan["cmbw"][e][:, None] * yb
    return acc


def _run(inputs, trace=False):
    from concourse import bass_utils
    plan = _plan(inputs)
    nc = _get_nc(plan["W"])
    res = bass_utils.run_bass_kernel_spmd(nc, plan["in_maps"],
                                          core_ids=list(range(NCORE)), trace=trace)
    return _combine(plan, res), res


def kernel(**inputs) -> np.ndarray:
    return _run(inputs, trace=False)[0]


# revision 16
# speedup vs baseline: 1.0992x; 1.0992x over previous
"""Ernie4 MoE (T=2048, H=1024, E=64 top-6, I=512 + shared SwiGLU MLP) on 8 Trainium2 cores.

Strategy: expert parallelism with host-mediated all-to-all.
  * Host computes the router (gate logits, sigmoid, top-6, renormalized
    combine weights) in fp32 and performs the dispatch: experts are ranked by
    routed token count and dealt round-robin to the 8 cores (rank r -> core
    r%8, slot r//8) so per-slot widths are balanced; the SPMD program uses
    slot widths W[s] = max token count over cores at slot s (16-aligned).
  * Expert gate/up weights and routed activations are quantized to fp8-e3m4
    with per-output-channel (resp. one global) scales; the scales are
    compensated exactly on device via the silu activation's per-partition
    scale operand, and by folding into w_down (bf16) on host. This halves
    the dominant weight stream (2MB/slot) and keeps rel err ~7e-3.
  * Every DRAM operand is prepacked on host into SBUF chunk layout
    ([128, contiguous bytes]) so each DMA is 128 fully-contiguous
    descriptors: weights one 2MB blob per slot on the scalar queue,
    activations on sync, outputs on gpsimd.
  * Device compute (per core, same program): shared SwiGLU MLP
    tensor-parallel over SI (slice of 128) + 8 expert SwiGLU FFNs on exact
    slot widths. Dense matmuls only - no on-device routing.
  * Outputs (shared partial and expert block, bf16) are combined on host:
    out = sum_c shared_c.T + combine-weighted scatter of y.
"""

import numpy as np
import ml_dtypes

T, H, E, K, I, SI = 2048, 1024, 64, 6, 512, 1024
NCORE = 8
EC = E // NCORE          # expert slots per core
SIC = SI // NCORE        # shared-intermediate slice per core
KC = H // 128            # hidden-dim 128-chunks
ICN = I // 128           # expert-intermediate 128-chunks
BF16 = ml_dtypes.bfloat16
F8 = ml_dtypes.float8_e3m4
F8MAX = 15.0

_CACHE = {}


def _build(W):
    import concourse.bass as bass
    import concourse.tile as tile
    from concourse import bacc, mybir

    f32 = mybir.dt.float32
    b16 = mybir.dt.bfloat16
    f8 = mybir.dt.float8e3
    u8 = mybir.dt.uint8
    AF = mybir.ActivationFunctionType
    OP = mybir.AluOpType

    WTOT = sum(W)
    Wmax = max(W)
    assert Wmax <= 512
    offs = np.concatenate([[0], np.cumsum(W)]).astype(int)
    WBUF = 5  # weight-blob prefetch depth (slots in flight)

    nc = bacc.Bacc("TRN2", target_bir_lowering=False, debug=False,
                   enable_asserts=False, num_devices=NCORE)

    xsl = nc.dram_tensor("xsl", [4, 128, KC * 512], b16, kind="ExternalInput").ap()
    xg = nc.dram_tensor("xg", [128, KC * WTOT], f8, kind="ExternalInput").ap()
    wall = nc.dram_tensor("wall", [EC, 128, 16384], u8, kind="ExternalInput").ap()
    wsh = nc.dram_tensor("wsh", [128, 3 * 1024], b16, kind="ExternalInput").ap()
    scv = nc.dram_tensor("scv", [128, EC * ICN], f32, kind="ExternalInput").ap()
    outp_sh = nc.dram_tensor("outp_sh", [4, 128, KC * 512], b16, kind="ExternalOutput").ap()
    yout = nc.dram_tensor("yout", [128, KC * WTOT], b16, kind="ExternalOutput").ap()

    with tile.TileContext(nc) as tc:
        with (
            tc.tile_pool(name="consts", bufs=1) as consts,
            tc.tile_pool(name="wpool", bufs=WBUF) as wpool,
            tc.tile_pool(name="xtpool", bufs=2) as xtpool,
            tc.tile_pool(name="spool", bufs=4) as spool,
            tc.tile_pool(name="apool", bufs=2) as apool,
            tc.tile_pool(name="shpool", bufs=2) as shpool,
            tc.tile_pool(name="ps", bufs=8, space="PSUM") as ps,
        ):
            # ---- expert weight blobs: one 2MB DMA per slot (scalar queue) ----
            wtiles = []

            def emit_wdma(s):
                wt = wpool.tile([128, 16384], u8, tag="wall")
                nc.scalar.dma_start(wt[:], wall[s])
                wtiles.append(wt)

            for s in range(WBUF):
                emit_wdma(s)

            def wviews(wt):
                wgv = wt[:, 0:4096].bitcast(f8).rearrange("p (kc i) -> p kc i", i=512)
                wuv = wt[:, 4096:8192].bitcast(f8).rearrange("p (kc i) -> p kc i", i=512)
                wdv = wt[:, 8192:16384].bitcast(b16).rearrange("p (ic h) -> p ic h", h=1024)
                return wgv, wuv, wdv

            # ---- per-slot routed-activation tiles (sync queue), fp8 ----
            xg_sbs = [consts.tile([128, KC * W[s]], f8, name=f"xg{s}")
                      for s in range(EC)]
            y_sbs = [consts.tile([128, KC * W[s]], b16, name=f"y{s}")
                     for s in range(EC)]

            def emit_xg(s):
                nc.sync.dma_start(xg_sbs[s][:], xg[:, KC * offs[s]:KC * offs[s + 1]])

            emit_xg(0)
            emit_xg(1)
            wsh_sb = consts.tile([128, 3 * 1024], b16)
            nc.sync.dma_start(wsh_sb[:], wsh)
            scv_sb = consts.tile([128, EC * ICN], f32)
            nc.sync.dma_start(scv_sb[:], scv)
            a_s = consts.tile([128, T], b16)

            def emit_shared_gateup(sl):
                xts = xtpool.tile([128, KC * 512], b16, tag="xts")
                nc.sync.dma_start(xts[:], xsl[sl])
                xv = xts[:].rearrange("p (kc t) -> p kc t", t=512)
                pg = ps.tile([128, 512], f32, tag="ps")
                for kc in range(KC):
                    nc.tensor.matmul(pg[:], wsh_sb[:, kc * 128:(kc + 1) * 128],
                                     xv[:, kc, :], start=(kc == 0), stop=(kc == KC - 1))
                pu = ps.tile([128, 512], f32, tag="ps")
                for kc in range(KC):
                    nc.tensor.matmul(pu[:], wsh_sb[:, 1024 + kc * 128:1024 + (kc + 1) * 128],
                                     xv[:, kc, :], start=(kc == 0), stop=(kc == KC - 1))
                sg = spool.tile([128, 512], f32, tag="sg")
                nc.scalar.activation(sg[:], pg[:], AF.Silu)
                nc.vector.tensor_tensor(a_s[:, sl * 512:(sl + 1) * 512], sg[:], pu[:], op=OP.mult)

            def emit_shared_down(sl):
                ysh = shpool.tile([128, KC * 512], b16, tag="ysh")
                for hc in range(KC):
                    py = ps.tile([128, 512], f32, tag="ps")
                    nc.tensor.matmul(py[:], wsh_sb[:, 2048 + hc * 128:2048 + (hc + 1) * 128],
                                     a_s[:, sl * 512:(sl + 1) * 512], start=True, stop=True)
                    if hc % 2 == 0:
                        nc.vector.tensor_copy(ysh[:, hc * 512:(hc + 1) * 512], py[:])
                    else:
                        nc.scalar.activation(ysh[:, hc * 512:(hc + 1) * 512], py[:], AF.Copy)
                nc.gpsimd.dma_start(outp_sh[sl], ysh[:])

            def emit_expert(s):
                w = W[s]
                wgv, wuv, wdv = wviews(wtiles[s])
                xv = xg_sbs[s][:].rearrange("p (kc w) -> p kc w", w=w)
                y_sb = y_sbs[s]
                a_sb = apool.tile([128, ICN, Wmax], b16, tag="a")
                for ic in range(ICN):
                    pg = ps.tile([128, 512], f32, tag="ps")
                    for kc in range(KC):
                        nc.tensor.matmul(pg[:, 0:w], wgv[:, kc, ic * 128:(ic + 1) * 128],
                                         xv[:, kc, :], start=(kc == 0), stop=(kc == KC - 1))
                    pu = ps.tile([128, 512], f32, tag="ps")
                    for kc in range(KC):
                        nc.tensor.matmul(pu[:, 0:w], wuv[:, kc, ic * 128:(ic + 1) * 128],
                                         xv[:, kc, :], start=(kc == 0), stop=(kc == KC - 1))
                    sg = spool.tile([128, 512], f32, tag="sg")
                    nc.scalar.activation(sg[:, 0:w], pg[:, 0:w], AF.Silu,
                                         scale=scv_sb[:, s * ICN + ic:s * ICN + ic + 1])
                    nc.vector.tensor_tensor(a_sb[:, ic, 0:w], sg[:, 0:w], pu[:, 0:w], op=OP.mult)

                for hc in range(KC):
                    py = ps.tile([128, 512], f32, tag="ps")
                    for ic in range(ICN):
                        nc.tensor.matmul(py[:, 0:w], wdv[:, ic, hc * 128:(hc + 1) * 128],
                                         a_sb[:, ic, 0:w], start=(ic == 0), stop=(ic == ICN - 1))
                    if hc % 2 == 0:
                        nc.vector.tensor_copy(y_sb[:, hc * w:(hc + 1) * w], py[:, 0:w])
                    else:
                        nc.scalar.activation(y_sb[:, hc * w:(hc + 1) * w], py[:, 0:w], AF.Copy)
                nc.gpsimd.dma_start(yout[:, KC * offs[s]:KC * offs[s + 1]], y_sb[:])
                # next weight blob: emitted AFTER this slot's compute so the
                # scalar engine's wait (tile release of slot s) cannot block
                # the silu ops the PE needs for slot s itself.
                if s + WBUF < EC:
                    emit_wdma(s + WBUF)

            # ---- schedule: experts 0-1 first (their weights land first and
            # their completion releases buffers for later blobs), shared MLP
            # as PE filler while blobs stream, then experts 2-7. ----
            emit_expert(0)
            for sl in range(4):
                emit_shared_gateup(sl)
            emit_expert(1)
            for s in range(2, EC):
                emit_xg(s)
            for sl in range(4):
                emit_shared_down(sl)
            for s in range(2, EC):
                emit_expert(s)

    nc.compile()
    return nc


def _get_nc(W):
    key = tuple(W)
    if key not in _CACHE:
        _CACHE[key] = _build(list(W))
    return _CACHE[key]


def _sbufize(m, nchunk):
    """[nchunk*128, F] row-major -> [128, nchunk*F] in SBUF chunk layout."""
    F = m.shape[1]
    return m.reshape(nchunk, 128, F).transpose(1, 0, 2).reshape(128, nchunk * F)


def _plan(inputs):
    x = np.ascontiguousarray(inputs["hidden_states"], dtype=np.float32)
    gate_w = np.asarray(inputs["gate_w"], dtype=np.float32)
    gate_bias = np.asarray(inputs["gate_bias"], dtype=np.float32)
    w_gate = np.asarray(inputs["w_gate"], dtype=np.float32)
    w_up = np.asarray(inputs["w_up"], dtype=np.float32)
    w_down = np.asarray(inputs["w_down"], dtype=np.float32)
    ws_gate = np.asarray(inputs["ws_gate"], dtype=np.float32)
    ws_up = np.asarray(inputs["ws_up"], dtype=np.float32)
    ws_down = np.asarray(inputs["ws_down"], dtype=np.float32)

    # ---- router (exact fp32, mirrors the reference) ----
    logits = x @ gate_w.T                                   # [T, E]
    scores = 1.0 / (1.0 + np.exp(-logits, dtype=np.float32))
    corrected = scores + gate_bias                          # [T, E]
    topk = np.argsort(-corrected, axis=1, kind="stable")[:, :K]  # [T, K]
    wsel = np.take_along_axis(scores, topk, axis=1)
    wsel = wsel / wsel.sum(axis=1, keepdims=True)

    toks = [None] * E
    cmbw = [None] * E
    sel = np.zeros((T, E), dtype=bool)
    sel[np.arange(T)[:, None], topk] = True
    wmat = np.zeros((T, E), dtype=np.float32)
    np.put_along_axis(wmat, topk, wsel, axis=1)
    for e in range(E):
        toks[e] = np.nonzero(sel[:, e])[0]
        cmbw[e] = wmat[toks[e], e]
    counts = np.array([len(t) for t in toks])

    # ---- balanced expert->(core,slot) assignment ----
    order = np.argsort(-counts, kind="stable")
    slot_expert = [[int(order[8 * s + c]) for s in range(EC)] for c in range(NCORE)]
    W = [max(16, int(-(-int(counts[order[8 * s]]) // 16) * 16)) for s in range(EC)]
    offs = np.concatenate([[0], np.cumsum(W)]).astype(int)
    WTOT = int(offs[-1])

    # xT in SBUF chunk layout [128, KC, T], then per-slab contiguous blocks
    xchunk = x.T.reshape(KC, 128, T).transpose(1, 0, 2)     # [128, KC, T]
    xsl = np.ascontiguousarray(
        xchunk.reshape(128, KC, 4, 512).transpose(2, 0, 1, 3).reshape(4, 128, KC * 512)
    ).astype(BF16)

    # global fp8 scale for x; quantize once, token-major
    sx = float(np.abs(x).max()) / F8MAX
    xq = (x / sx).astype(F8)                                # [T, H] fp8

    in_maps = []
    for c in range(NCORE):
        es = slot_expert[c]
        xg = np.zeros((128, KC * WTOT), dtype=F8)
        wall = np.empty((EC, 128, 16384), dtype=np.uint8)
        scvm = np.empty((128, EC * ICN), dtype=np.float32)
        for s in range(EC):
            e = es[s]
            tk = toks[e]
            blk = np.zeros((128, KC, W[s]), dtype=F8)
            xt = np.ascontiguousarray(xq[tk].T)             # [H, cnt] fp8
            blk[:, :, 0:len(tk)] = xt.reshape(KC, 128, len(tk)).transpose(1, 0, 2)
            xg[:, KC * offs[s]:KC * offs[s + 1]] = blk.reshape(128, -1)
            wg_ = w_gate[e]
            wu_ = w_up[e]
            sgv = np.abs(wg_).max(axis=0) / F8MAX + 1e-30   # [I]
            suv = np.abs(wu_).max(axis=0) / F8MAX + 1e-30
            wgq = (wg_ / sgv).astype(F8)
            wuq = (wu_ / suv).astype(F8)
            wd2 = (w_down[e] * (suv * sx)[:, None]).astype(BF16)
            wall[s, :, 0:4096] = _sbufize(wgq, KC).view(np.uint8)
            wall[s, :, 4096:8192] = _sbufize(wuq, KC).view(np.uint8)
            wall[s, :, 8192:16384] = _sbufize(wd2, ICN).view(np.uint8)
            scvm[:, s * ICN:(s + 1) * ICN] = (sgv * sx).reshape(ICN, 128).T
        wsh = np.concatenate([
            _sbufize(ws_gate[:, c * SIC:(c + 1) * SIC], KC),
            _sbufize(ws_up[:, c * SIC:(c + 1) * SIC], KC),
            ws_down[c * SIC:(c + 1) * SIC, :],
        ], axis=1)                                          # [128, 3*1024]
        in_maps.append({
            "xsl": xsl,
            "xg": xg,
            "wall": wall,
            "wsh": wsh.astype(BF16),
            "scv": scvm,
        })
    return {"W": W, "offs": offs, "slot_expert": slot_expert, "toks": toks,
            "cmbw": cmbw, "in_maps": in_maps}


def _combine(plan, res):
    acc = np.zeros((T, H), dtype=np.float32)
    offs, W = plan["offs"], plan["W"]
    for c in range(NCORE):
        r = res.results[c]
        # outp_sh: [4, 128, KC*512] slab-chunk layout -> [H, T] -> add as [T, H]
        sh = np.asarray(r["outp_sh"]).astype(np.float32).reshape(4, 128, KC, 512)
        acc += sh.transpose(2, 1, 0, 3).reshape(H, T).T
        y = np.asarray(r["yout"]).astype(np.float32)        # [128, KC*WTOT]
        for s in range(EC):
            e = plan["slot_expert"][c][s]
            tk = plan["toks"][e]
            if len(tk) == 0:
                continue
            yb = y[:, KC * offs[s]:KC * offs[s + 1]].reshape(128, KC, W[s])
            yb = yb.transpose(1, 0, 2).reshape(H, W[s])[:, 0:len(tk)]
            acc[tk, :] += plan["cmbw"][e][:, None] * yb.T
    return acc


def _run(inputs, trace=False):
    from concourse import bass_utils
    plan = _plan(inputs)
    nc = _get_nc(plan["W"])
    res = bass_utils.run_bass_kernel_spmd(nc, plan["in_maps"],
                                          core_ids=list(range(NCORE)), trace=trace)
    return _combine(plan, res), res


def kernel(**inputs) -> np.ndarray:
    return _run(inputs, trace=False)[0]
